# Initial kernel scaffold
#
"""GNN edge-softmax attention kernel for Trainium2 (8 NeuronCores).

Node-partitioned, degree-grouped, lane-major layout:
  - nodes are partitioned across the 8 cores by target-node range; every edge
    lives on the core that owns its target node, so no collectives are needed.
  - within a core, nodes are sorted by degree and grouped into tiles of 128;
    each tile is padded to a uniform capacity C_t (its max degree across all
    cores, so the SPMD graph is shape-identical on every core).
  - slot layout is lane-major: edge j of the node on lane l sits at
    (chunk j, partition l).  Segment softmax reductions then become free-dim
    reductions, and the per-edge gather of the target node's query projection
    becomes an identity matmul against the resident g'' table.

All the algebra that involves only weights is folded on the host:
  scores = W2 @ relu(W1a@q_i + W1b@k + b1) + b2  with q_i = Wq@x[tgt]
         = sum_m sgn_m * relu(z_m) + b2,   z = |W2|*(W1b)@k + g''[tgt]
  where g'' = x @ (|W2|*W1a @ Wq).T + |W2|*b1 is computed on device per node.
"""

import os
import sys
import numpy as np

sys.path.insert(0, "/opt/trn_rl_repo")

N_NODES = 100000
N_EDGES = 1000000
D = 128
EDGE_DIM = 10
NCORES = 8
NPC = N_NODES // NCORES          # 12500 nodes per core
LANES = 128
NTILES = (NPC + LANES - 1) // LANES   # 98
NPAD = NTILES * LANES                 # 12544
ALPHA = 0.01
EPS = 1e-16
NEGINF = -1.0e30


# ---------------------------------------------------------------------------
# host-side preparation
# ---------------------------------------------------------------------------

def _schedule(tgt):
    """Compute the shared per-tile capacity schedule and per-core layouts."""
    per_core = []
    md_all = np.zeros((NCORES, NTILES), dtype=np.int64)
    for c in range(NCORES):
        ids = np.nonzero((tgt >= c * NPC) & (tgt < (c + 1) * NPC))[0]
        tl = tgt[ids] - c * NPC
        deg = np.bincount(tl, minlength=NPC)
        degp = np.concatenate([deg, np.zeros(NPAD - NPC, dtype=deg.dtype)])
        order = np.argsort(degp, kind="stable")        # ascending degree
        md_all[c] = degp[order].reshape(NTILES, LANES).max(axis=1)
        per_core.append((ids, tl, degp, order))
    C = np.maximum(md_all.max(axis=0), 1).astype(np.int64)   # per-tile capacity
    chunk_base = np.concatenate([[0], np.cumsum(C)])         # in chunks of 128
    return C, chunk_base, per_core


def _layout_core(ids, tl, degp, order, C, chunk_base, ea, x, core):
    """Build the formatted per-core device inputs + recovery indices."""
    TC = int(chunk_base[-1])
    S = TC * LANES
    inv = np.empty(NPAD, dtype=np.int64)
    inv[order] = np.arange(NPAD)

    # rank of each edge within its target node (in original edge order)
    n_loc = tl
    ord_e = np.argsort(n_loc, kind="stable")
    sorted_nodes = n_loc[ord_e]
    starts = np.searchsorted(sorted_nodes, np.arange(NPC))
    rank_sorted = np.arange(len(n_loc)) - starts[sorted_nodes]
    rank = np.empty(len(n_loc), dtype=np.int64)
    rank[ord_e] = rank_sorted

    p = inv[n_loc]                       # sorted position of the node
    t = p // LANES                       # tile
    lane = p % LANES
    chunk = chunk_base[t] + rank         # global chunk index
    slot = chunk * LANES + lane

    ea_T = np.zeros((EDGE_DIM + 1, S), dtype=np.float16)
    ea_T[:EDGE_DIM, slot] = ea[ids].T.astype(np.float16)
    ea_T[EDGE_DIM, :] = np.float16(1.0)

    mask = np.full((LANES, TC), NEGINF, dtype=np.float32)
    mask[lane, chunk] = 0.0

    xg = np.zeros((NPAD, D), dtype=np.float32)
    sel = order < NPC
    xg[sel] = x[core * NPC + order[sel]]
    x_T = np.ascontiguousarray(xg.T).astype(np.float16)

    return {
        "ea_T": ea_T, "mask": mask, "x_T": x_T,
        "ids": ids, "lane": lane, "chunk": chunk, "order": order,
    }


def _prep_weights(Wq, Wk, Wv, bv, W1, b1, W2, b2):
    W2v = W2[0].astype(np.float64)
    pos = W2v > 0
    P = int(pos.sum())
    perm = np.concatenate([np.nonzero(pos)[0], np.nonzero(~pos)[0]])
    cm = np.abs(W2v)[perm]
    W1a_s = (np.abs(W2v)[:, None] * W1[:, :D].astype(np.float64))[perm]
    W1b_s = (np.abs(W2v)[:, None] * W1[:, D:].astype(np.float64))[perm]
    b1_s = (np.abs(W2v) * b1.astype(np.float64))[perm]

    f16 = lambda a: np.ascontiguousarray(a).astype(np.float16)
    return {
        "P": P,
        "b2": float(b2[0]),
        "WkT_aug": f16(np.vstack([Wk.T, np.zeros((1, D))])),        # [11,128]
        "WvT_aug": f16(np.vstack([Wv.T, bv[None, :]])),             # [11,128]
        "W1bT_s": f16(W1b_s.T),                                     # [d,m]
        "Wq_l": f16(Wq),                                            # [t,d] lhsT
        "W1aT_s": f16(W1a_s.T),                                     # [t,m] rhs
        "b1_row": f16(b1_s[None, :]),                               # [1,128]
    }


# ---------------------------------------------------------------------------
# device graph
# ---------------------------------------------------------------------------

def build_graph(C, P, debug=False):
    """Build the SPMD Bass graph shared by all cores.

    C: per-tile capacities (len NTILES).  P: positive-sign split of the m dim.
    """
    from concourse import bacc, bass, tile
    from concourse import mybir

    f16 = mybir.dt.float16
    f32 = mybir.dt.float32
    AF = mybir.ActivationFunctionType
    ALU = mybir.AluOpType
    AX = mybir.AxisListType

    ntiles = len(C)
    TC = int(np.sum(C))
    S = TC * LANES

    nc = bacc.Bacc(None, target_bir_lowering=False, debug=debug)

    ea_T = nc.declare_dram_parameter("ea_T", [EDGE_DIM + 1, S], f16).ap()
    mask = nc.declare_dram_parameter("mask", [LANES, TC], f32).ap()
    x_T = nc.declare_dram_parameter("x_T", [D, NPAD], f16).ap()
    WkT = nc.declare_dram_parameter("WkT_aug", [EDGE_DIM + 1, D], f16).ap()
    WvT = nc.declare_dram_parameter("WvT_aug", [EDGE_DIM + 1, D], f16).ap()
    W1bT = nc.declare_dram_parameter("W1bT_s", [D, D], f16).ap()
    Wq_l = nc.declare_dram_parameter("Wq_l", [D, D], f16).ap()
    W1aT = nc.declare_dram_parameter("W1aT_s", [D, D], f16).ap()
    b1_row = nc.declare_dram_parameter("b1_row", [1, D], f16).ap()

    outp = nc.declare_dram_parameter("outp", [NPAD, D], f32, isOutput=True).ap()
    attn_o = nc.declare_dram_parameter("attn_o", [LANES, TC], f32, isOutput=True).ap()

    with tile.TileContext(nc) as tc:
        with (
            tc.tile_pool(name="const", bufs=1) as cpool,
            tc.tile_pool(name="big", bufs=1) as bigpool,
            tc.tile_pool(name="work", bufs=3) as wpool,
            tc.tile_pool(name="ev", bufs=3) as evpool,
            tc.tile_pool(name="psum", bufs=2, space="PSUM") as pspool,
            tc.tile_pool(name="psacc", bufs=2, space="PSUM") as psacc,
        ):
            # ---- constants to SBUF ----
            def sload(ap, shape, dtype):
                t = cpool.tile(shape, dtype)
                nc.sync.dma_start(out=t[:], in_=ap[:])
                return t

            WkT_sb = sload(WkT, [EDGE_DIM + 1, D], f16)
            WvT_sb = sload(WvT, [EDGE_DIM + 1, D], f16)
            W1bT_sb = sload(W1bT, [D, D], f16)
            Wq_sb = sload(Wq_l, [D, D], f16)
            W1aT_sb = sload(W1aT, [D, D], f16)
            b1_sb = sload(b1_row, [1, D], f16)
            mask_sb = sload(mask, [LANES, TC], f32)

            ones_sb = cpool.tile([1, D], f16)
            nc.vector.memset(ones_sb[:], 1.0)
            ident_sb = cpool.tile([D, D], f16)
            nc.vector.memset(ident_sb[:], 0.0)
            # identity via iota trick: use gpsimd memset + per-partition writes
            # is awkward; build identity with iota + select on DVE instead.
            iota_p = cpool.tile([D, 1], f32)
            nc.vector.iota(iota_p[:], pattern=[[1, D]], base=0, channel_multiplier=1)
            iota_f = cpool.tile([D, D], f32)
            nc.vector.iota(iota_f[:], pattern=[[1, D]], base=0, channel_multiplier=0)
            eqm = cpool.tile([D, D], f32)
            nc.vector.tensor_scalar(
                out=eqm[:], in0=iota_f[:], scalar1=iota_p[:], scalar2=None,
                op0=ALU.is_equal,
            )
            nc.vector.tensor_copy(ident_sb[:], eqm[:])

            x_sb = bigpool.tile([D, NPAD], f16)
            nc.sync.dma_start(out=x_sb[:], in_=x_T[:])

            # ---- A'T = Wq.T @ W1a_s.T  (A' = W1a_s @ Wq), then g'' table ----
            psA = pspool.tile([D, D], f32, tag="psk")
            nc.tensor.matmul(psA[:], lhsT=Wq_sb[:], rhs=W1aT_sb[:], start=True, stop=True)
            A_sb = cpool.tile([D, D], f16)
            nc.scalar.activation(A_sb[:], psA[:], AF.Copy)

            g_sb = bigpool.tile([D, ntiles * LANES], f16)   # [node-lane, tile*m]
            for t in range(ntiles):
                psG = pspool.tile([D, D], f32, tag="psk")
                nc.tensor.matmul(
                    psG[:], lhsT=x_sb[:, t * LANES:(t + 1) * LANES], rhs=A_sb[:],
                    start=True, stop=False)
                nc.tensor.matmul(
                    psG[:], lhsT=ones_sb[:], rhs=b1_sb[:], start=False, stop=True)
                nc.scalar.activation(
                    g_sb[:, t * LANES:(t + 1) * LANES], psG[:], AF.Copy)

            # ---- main loop over node tiles ----
            attn_sb = bigpool.tile([LANES, TC], f32)
            cb = 0
            for t in range(ntiles):
                Ct = int(C[t])
                ea_t = wpool.tile([EDGE_DIM + 1, Ct * LANES], f16, tag="ea")
                nc.sync.dma_start(
                    out=ea_t[:], in_=ea_T[:, cb * LANES:(cb + Ct) * LANES])

                kT_t = wpool.tile([D, Ct * LANES], f16, tag="kT")
                vst = wpool.tile([D, Ct * LANES], f16, tag="vst")
                hA = wpool.tile([D, Ct * LANES], f16, tag="hA")
                accA = wpool.tile([LANES, Ct], f32, tag="accA")
                accB = wpool.tile([LANES, Ct], f32, tag="accB")

                g_blk = g_sb[:, t * LANES:(t + 1) * LANES]

                for q0 in range(0, Ct, 4):
                    w = min(4, Ct - q0)
                    cols = w * LANES
                    sl = slice(q0 * LANES, q0 * LANES + cols)

                    # k = prelu(ea @ WkT) in d-major [d, e]
                    psK = pspool.tile([D, 512], f32, tag="psk")
                    nc.tensor.matmul(
                        psK[:, :cols], lhsT=WkT_sb[:], rhs=ea_t[:, sl],
                        start=True, stop=True)
                    nc.scalar.activation(
                        kT_t[:, sl], psK[:, :cols], AF.Prelu, alpha=ALPHA)

                    # v = prelu(ea @ WvT + bv) in lane-major [e, d]
                    psV = pspool.tile([D, 512], f32, tag="psv")
                    for i in range(w):
                        c0 = (q0 + i) * LANES
                        nc.tensor.matmul(
                            psV[:, i * LANES:(i + 1) * LANES],
                            lhsT=ea_t[:, c0:c0 + LANES], rhs=WvT_sb[:],
                            start=True, stop=True)
                    nc.scalar.activation(
                        vst[:, sl], psV[:, :cols], AF.Prelu, alpha=ALPHA)

                    # z = k @ W1b'.T + g''[lane]  in lane-major [e, m]
                    psZ = pspool.tile([D, 512], f32, tag="psz")
                    for i in range(w):
                        c0 = (q0 + i) * LANES
                        zsl = slice(i * LANES, (i + 1) * LANES)
                        nc.tensor.matmul(
                            psZ[:, zsl], lhsT=kT_t[:, c0:c0 + LANES],
                            rhs=W1bT_sb[:], start=True, stop=False)
                        nc.tensor.matmul(
                            psZ[:, zsl], lhsT=ident_sb[:], rhs=g_blk,
                            start=False, stop=True)
                    # h = relu(z) (fp16), then signed free-dim reduces
                    nc.scalar.activation(hA[:, sl], psZ[:, :cols], AF.Relu)
                    hv = hA[:, sl].reshape([D, w, LANES])
                    nc.vector.tensor_reduce(
                        out=accA[:, q0:q0 + w], in_=hv[:, :, :P],
                        axis=AX.X, op=ALU.add)
                    nc.vector.tensor_reduce(
                        out=accB[:, q0:q0 + w], in_=hv[:, :, P:],
                        axis=AX.X, op=ALU.add)

                # scores -> e (exp), masked; then denom, attn
                s_t = evpool.tile([LANES, Ct], f32, tag="s")
                nc.vector.tensor_tensor(
                    out=s_t[:], in0=accA[:], in1=accB[:], op=ALU.subtract)
                nc.vector.tensor_tensor(
                    out=s_t[:], in0=s_t[:], in1=mask_sb[:, cb:cb + Ct], op=ALU.add)
                e_t = evpool.tile([LANES, Ct], f32, tag="e")
                nc.scalar.activation(e_t[:], s_t[:], AF.Exp)

                den = evpool.tile([LANES, 1], f32, tag="den")
                nc.vector.tensor_reduce(out=den[:], in_=e_t[:], axis=AX.X, op=ALU.add)
                rec = evpool.tile([LANES, 1], f32, tag="rec")
                nc.vector.tensor_scalar(
                    out=den[:], in0=den[:], scalar1=EPS, scalar2=None, op0=ALU.add)
                nc.vector.reciprocal(rec[:], den[:])
                nc.vector.tensor_scalar(
                    out=attn_sb[:, cb:cb + Ct], in0=e_t[:], scalar1=rec[:],
                    scalar2=None, op0=ALU.mult)

                # out = sum_j e_j * v_j  via diag(e) matmuls, then normalize
                psO = psacc.tile([D, D], f32, tag="pso")
                for j in range(Ct):
                    dg = evpool.tile([D, D], f16, tag="diag")
                    nc.vector.tensor_scalar(
                        out=dg[:], in0=ident_sb[:], scalar1=e_t[:, j:j + 1],
                        scalar2=None, op0=ALU.mult)
                    nc.tensor.matmul(
                        psO[:], lhsT=dg[:], rhs=vst[:, j * LANES:(j + 1) * LANES],
                        start=(j == 0), stop=(j == Ct - 1))
                out_t = evpool.tile([D, D], f32, tag="outt")
                nc.vector.tensor_scalar(
                    out=out_t[:], in0=psO[:], scalar1=rec[:], scalar2=None,
                    op0=ALU.mult)
                nc.sync.dma_start(
                    out=outp[t * LANES:(t + 1) * LANES, :], in_=out_t[:])

                cb += Ct

            nc.sync.dma_start(out=attn_o[:], in_=attn_sb[:])

    nc.compile()
    return nc


# ---------------------------------------------------------------------------
# entry point
# ---------------------------------------------------------------------------

def kernel(x, edge_index, edge_attr, Wq, Wk, Wv, bv, W1, b1, W2, b2):
    x = np.asarray(x, dtype=np.float32)
    edge_index = np.asarray(edge_index)
    ea = np.asarray(edge_attr, dtype=np.float32)
    Wq = np.asarray(Wq, np.float32); Wk = np.asarray(Wk, np.float32)
    Wv = np.asarray(Wv, np.float32); bv = np.asarray(bv, np.float32)
    W1 = np.asarray(W1, np.float32); b1 = np.asarray(b1, np.float32)
    W2 = np.asarray(W2, np.float32); b2 = np.asarray(b2, np.float32)

    tgt = edge_index[1].astype(np.int64)
    C, chunk_base, per_core = _schedule(tgt)
    wts = _prep_weights(Wq, Wk, Wv, bv, W1, b1, W2, b2)

    cores = []
    for c in range(NCORES):
        ids, tl, degp, order = per_core[c]
        cores.append(_layout_core(ids, tl, degp, order, C, chunk_base, ea, x, c))

    nc = build_graph(C, wts["P"])

    in_maps = []
    for c in range(NCORES):
        lay = cores[c]
        m = lay["mask"] + np.float32(wts["b2"])
        # padding slots keep NEGINF (+b2 is absorbed: NEGINF + b2 ~ NEGINF)
        m[lay["mask"] < -1e29] = NEGINF
        in_maps.append({
            "ea_T": lay["ea_T"],
            "mask": m.astype(np.float32),
            "x_T": lay["x_T"],
            "WkT_aug": wts["WkT_aug"],
            "WvT_aug": wts["WvT_aug"],
            "W1bT_s": wts["W1bT_s"],
            "Wq_l": wts["Wq_l"],
            "W1aT_s": wts["W1aT_s"],
            "b1_row": wts["b1_row"],
        })

    from concourse.bass_utils import run_bass_kernel_spmd
    res = run_bass_kernel_spmd(nc, in_maps, core_ids=list(range(NCORES)))

    out = np.zeros((N_NODES, D), dtype=np.float32)
    attn = np.zeros(N_EDGES, dtype=np.float32)
    for c in range(NCORES):
        lay = cores[c]
        r = res.results[c]
        dev_out = r["outp"]          # [NPAD, 128] in sorted-node order
        dev_attn = r["attn_o"]       # [128, TC]
        order = lay["order"]
        sel = order < NPC
        out[c * NPC + order[sel]] = dev_out[sel]
        attn[lay["ids"]] = dev_attn[lay["lane"], lay["chunk"]]

    return out, attn


if __name__ == "__main__":
    pass


# revision 8
# speedup vs baseline: 1.0592x; 1.0592x over previous
"""GNN edge-softmax attention kernel for Trainium2 (8 NeuronCores).

Node-partitioned, degree-grouped, lane-major layout:
  - nodes are partitioned across the 8 cores by target-node range; every edge
    lives on the core that owns its target node, so no collectives are needed.
  - within a core, nodes are sorted by degree and grouped into tiles of 128;
    each tile is padded to a uniform capacity C_t (its max degree across all
    cores, so the SPMD graph is shape-identical on every core).
  - slot layout is lane-major: edge j of the node on lane l sits at
    (chunk j, partition l).  Segment softmax reductions then become free-dim
    reductions, and the per-edge gather of the target node's query projection
    becomes an identity matmul against the resident g'' table.

All the algebra that involves only weights is folded on the host:
  scores = W2 @ relu(W1a@q_i + W1b@k + b1) + b2  with q_i = Wq@x[tgt]
         = sum_m sgn_m * relu(z_m) + b2,   z = |W2|*(W1b)@k + g''[tgt]
  where g'' = x @ (|W2|*W1a @ Wq).T + |W2|*b1 is computed on device per node.
"""

import os
import sys
import numpy as np

try:
    import concourse  # noqa: F401  (provided by the environment)
except ImportError:
    sys.path.append("/opt/trn_rl_repo")

N_NODES = 100000
N_EDGES = 1000000
D = 128
EDGE_DIM = 10
NCORES = 8
NPC = N_NODES // NCORES          # 12500 nodes per core
LANES = 128
NTILES = (NPC + LANES - 1) // LANES   # 98
NPAD = NTILES * LANES                 # 12544
ALPHA = 0.01
EPS = 1e-16
NEGINF = -1.0e30


# ---------------------------------------------------------------------------
# host-side preparation
# ---------------------------------------------------------------------------

def _schedule(tgt):
    """Compute the shared per-tile capacity schedule and per-core layouts."""
    per_core = []
    md_all = np.zeros((NCORES, NTILES), dtype=np.int64)
    for c in range(NCORES):
        ids = np.nonzero((tgt >= c * NPC) & (tgt < (c + 1) * NPC))[0]
        tl = tgt[ids] - c * NPC
        deg = np.bincount(tl, minlength=NPC)
        degp = np.concatenate([deg, np.zeros(NPAD - NPC, dtype=deg.dtype)])
        order = np.argsort(degp, kind="stable")        # ascending degree
        md_all[c] = degp[order].reshape(NTILES, LANES).max(axis=1)
        per_core.append((ids, tl, degp, order))
    C = np.maximum(md_all.max(axis=0), 1).astype(np.int64)   # per-tile capacity
    chunk_base = np.concatenate([[0], np.cumsum(C)])         # in chunks of 128
    return C, chunk_base, per_core


def _layout_core(ids, tl, degp, order, C, chunk_base, ea, x, core):
    """Build the formatted per-core device inputs + recovery indices."""
    TC = int(chunk_base[-1])
    S = TC * LANES
    inv = np.empty(NPAD, dtype=np.int64)
    inv[order] = np.arange(NPAD)

    # rank of each edge within its target node (in original edge order)
    n_loc = tl
    ord_e = np.argsort(n_loc, kind="stable")
    sorted_nodes = n_loc[ord_e]
    starts = np.searchsorted(sorted_nodes, np.arange(NPC))
    rank_sorted = np.arange(len(n_loc)) - starts[sorted_nodes]
    rank = np.empty(len(n_loc), dtype=np.int64)
    rank[ord_e] = rank_sorted

    p = inv[n_loc]                       # sorted position of the node
    t = p // LANES                       # tile
    lane = p % LANES
    chunk = chunk_base[t] + rank         # global chunk index
    slot = chunk * LANES + lane

    ea_T = np.zeros((EDGE_DIM + 1, S), dtype=np.float16)
    ea_T[:EDGE_DIM, slot] = ea[ids].T.astype(np.float16)
    ea_T[EDGE_DIM, :] = np.float16(1.0)

    mask = np.full((LANES, TC), NEGINF, dtype=np.float32)
    mask[lane, chunk] = 0.0

    xg = np.zeros((NPAD, D), dtype=np.float32)
    sel = order < NPC
    xg[sel] = x[core * NPC + order[sel]]
    x_T = np.ascontiguousarray(xg.T).astype(np.float16)

    return {
        "ea_T": ea_T, "mask": mask, "x_T": x_T,
        "ids": ids, "lane": lane, "chunk": chunk, "order": order,
    }


def _prep_weights(Wq, Wk, Wv, bv, W1, b1, W2, b2):
    W2v = W2[0].astype(np.float64)
    pos = W2v > 0
    P = int(pos.sum())
    perm = np.concatenate([np.nonzero(pos)[0], np.nonzero(~pos)[0]])
    cm = np.abs(W2v)[perm]
    W1a_s = (np.abs(W2v)[:, None] * W1[:, :D].astype(np.float64))[perm]
    W1b_s = (np.abs(W2v)[:, None] * W1[:, D:].astype(np.float64))[perm]
    b1_s = (np.abs(W2v) * b1.astype(np.float64))[perm]

    f16 = lambda a: np.ascontiguousarray(a).astype(np.float16)
    return {
        "P": P,
        "b2": float(b2[0]),
        "WkT_aug": f16(np.vstack([Wk.T, np.zeros((1, D))])),        # [11,128]
        "WvT_aug": f16(np.vstack([Wv.T, bv[None, :]])),             # [11,128]
        "W1bT_s": f16(W1b_s.T),                                     # [d,m]
        "Wq_l": f16(Wq),                                            # [t,d] lhsT
        "W1aT_s": f16(W1a_s.T),                                     # [t,m] rhs
        "b1_row": f16(b1_s[None, :]),                               # [1,128]
    }


# ---------------------------------------------------------------------------
# device graph
# ---------------------------------------------------------------------------

def build_graph(C, P, debug=False):
    """Build the SPMD Bass graph shared by all cores.

    C: per-tile capacities (len NTILES).  P: positive-sign split of the m dim.
    """
    from concourse import bacc, bass, tile
    from concourse import mybir

    f16 = mybir.dt.float16
    f32 = mybir.dt.float32
    AF = mybir.ActivationFunctionType
    ALU = mybir.AluOpType
    AX = mybir.AxisListType

    ntiles = len(C)
    TC = int(np.sum(C))
    S = TC * LANES

    nc = bacc.Bacc(None, target_bir_lowering=False, debug=debug)

    ea_T = nc.declare_dram_parameter("ea_T", [EDGE_DIM + 1, S], f16, isOutput=False)
    mask = nc.declare_dram_parameter("mask", [LANES, TC], f32, isOutput=False)
    x_T = nc.declare_dram_parameter("x_T", [D, NPAD], f16, isOutput=False)
    WkT = nc.declare_dram_parameter("WkT_aug", [EDGE_DIM + 1, D], f16, isOutput=False)
    WvT = nc.declare_dram_parameter("WvT_aug", [EDGE_DIM + 1, D], f16, isOutput=False)
    W1bT = nc.declare_dram_parameter("W1bT_s", [D, D], f16, isOutput=False)
    Wq_l = nc.declare_dram_parameter("Wq_l", [D, D], f16, isOutput=False)
    W1aT = nc.declare_dram_parameter("W1aT_s", [D, D], f16, isOutput=False)
    b1_row = nc.declare_dram_parameter("b1_row", [1, D], f16, isOutput=False)
    ident = nc.declare_dram_parameter("ident", [D, D], f16, isOutput=False)

    outp = nc.declare_dram_parameter("outp", [len(C) * LANES, D], f32, isOutput=True)
    attn_o = nc.declare_dram_parameter("attn_o", [LANES, TC], f32, isOutput=True)

    with tile.TileContext(nc) as tc:
        with (
            tc.tile_pool(name="const", bufs=1) as cpool,
            tc.tile_pool(name="big", bufs=1) as bigpool,
            tc.tile_pool(name="work", bufs=3) as wpool,
            tc.tile_pool(name="ev", bufs=3) as evpool,
            tc.tile_pool(name="psum", bufs=2, space="PSUM") as pspool,
            tc.tile_pool(name="psacc", bufs=2, space="PSUM") as psacc,
        ):
            # ---- constants to SBUF ----
            def sload(ap, shape, dtype, tag):
                t = cpool.tile(shape, dtype, tag=tag)
                nc.sync.dma_start(out=t[:], in_=ap[:])
                return t

            WkT_sb = sload(WkT, [EDGE_DIM + 1, D], f16, "c_wkt")
            WvT_sb = sload(WvT, [EDGE_DIM + 1, D], f16, "c_wvt")
            W1bT_sb = sload(W1bT, [D, D], f16, "c_w1bt")
            Wq_sb = sload(Wq_l, [D, D], f16, "c_wq")
            W1aT_sb = sload(W1aT, [D, D], f16, "c_w1at")
            b1_sb = sload(b1_row, [1, D], f16, "c_b1")
            mask_sb = sload(mask, [LANES, TC], f32, "c_mask")

            ones_sb = cpool.tile([1, D], f16, tag="c_ones")
            nc.vector.memset(ones_sb[:], 1.0)
            ident_sb = sload(ident, [D, D], f16, "c_ident")

            x_sb = bigpool.tile([D, NPAD], f16)
            nc.sync.dma_start(out=x_sb[:], in_=x_T[:])

            # ---- A'T = Wq.T @ W1a_s.T  (A' = W1a_s @ Wq), then g'' table ----
            psA = pspool.tile([D, D], f32, tag="psk")
            nc.tensor.matmul(psA[:], lhsT=Wq_sb[:], rhs=W1aT_sb[:], start=True, stop=True)
            A_sb = cpool.tile([D, D], f16)
            nc.scalar.activation(A_sb[:], psA[:], AF.Copy)

            g_sb = bigpool.tile([D, ntiles * LANES], f16)   # [node-lane, tile*m]
            for t in range(ntiles):
                psG = pspool.tile([D, D], f32, tag="psk")
                nc.tensor.matmul(
                    psG[:], lhsT=x_sb[:, t * LANES:(t + 1) * LANES], rhs=A_sb[:],
                    start=True, stop=False)
                nc.tensor.matmul(
                    psG[:], lhsT=ones_sb[:], rhs=b1_sb[:], start=False, stop=True)
                nc.scalar.activation(
                    g_sb[:, t * LANES:(t + 1) * LANES], psG[:], AF.Copy)

            # ---- main loop over node tiles ----
            attn_sb = bigpool.tile([LANES, TC], f32)
            cb = 0
            for t in range(ntiles):
                Ct = int(C[t])
                ea_t = wpool.tile([EDGE_DIM + 1, Ct * LANES], f16, tag="ea")
                nc.sync.dma_start(
                    out=ea_t[:], in_=ea_T[:, cb * LANES:(cb + Ct) * LANES])

                kT_t = wpool.tile([D, Ct * LANES], f16, tag="kT")
                vst = wpool.tile([D, Ct * LANES], f16, tag="vst")
                hA = wpool.tile([D, Ct, LANES], f16, tag="hA")
                accA = wpool.tile([LANES, Ct], f32, tag="accA")
                accB = wpool.tile([LANES, Ct], f32, tag="accB")

                g_blk = g_sb[:, t * LANES:(t + 1) * LANES]

                for q0 in range(0, Ct, 4):
                    w = min(4, Ct - q0)
                    cols = w * LANES
                    sl = slice(q0 * LANES, q0 * LANES + cols)

                    # k = prelu(ea @ WkT) in d-major [d, e]
                    psK = pspool.tile([D, 512], f32, tag="psk")
                    nc.tensor.matmul(
                        psK[:, :cols], lhsT=WkT_sb[:], rhs=ea_t[:, sl],
                        start=True, stop=True)
                    nc.scalar.activation(
                        kT_t[:, sl], psK[:, :cols], AF.Prelu, alpha=ALPHA)

                    # v = prelu(ea @ WvT + bv) in lane-major [e, d]
                    psV = pspool.tile([D, 512], f32, tag="psv")
                    for i in range(w):
                        c0 = (q0 + i) * LANES
                        nc.tensor.matmul(
                            psV[:, i * LANES:(i + 1) * LANES],
                            lhsT=ea_t[:, c0:c0 + LANES], rhs=WvT_sb[:],
                            start=True, stop=True)
                    nc.scalar.activation(
                        vst[:, sl], psV[:, :cols], AF.Prelu, alpha=ALPHA)

                    # z = k @ W1b'.T + g''[lane]  in lane-major [e, m]
                    psZ = pspool.tile([D, 4, LANES], f32, tag="psz")
                    for i in range(w):
                        c0 = (q0 + i) * LANES
                        nc.tensor.matmul(
                            psZ[:, i, :], lhsT=kT_t[:, c0:c0 + LANES],
                            rhs=W1bT_sb[:], start=True, stop=False)
                        nc.tensor.matmul(
                            psZ[:, i, :], lhsT=ident_sb[:], rhs=g_blk,
                            start=False, stop=True)
                    # h = relu(z) (fp16), then signed free-dim reduces
                    nc.scalar.activation(
                        hA[:, q0:q0 + w, :], psZ[:, :w, :], AF.Relu)
                    nc.vector.tensor_reduce(
                        out=accA[:, q0:q0 + w], in_=hA[:, q0:q0 + w, :P],
                        axis=AX.X, op=ALU.add)
                    nc.vector.tensor_reduce(
                        out=accB[:, q0:q0 + w], in_=hA[:, q0:q0 + w, P:],
                        axis=AX.X, op=ALU.add)

                # scores -> e (exp), masked; then denom, attn
                s_t = evpool.tile([LANES, Ct], f32, tag="s")
                nc.vector.tensor_tensor(
                    out=s_t[:], in0=accA[:], in1=accB[:], op=ALU.subtract)
                nc.vector.tensor_tensor(
                    out=s_t[:], in0=s_t[:], in1=mask_sb[:, cb:cb + Ct], op=ALU.add)
                e_t = evpool.tile([LANES, Ct], f32, tag="e")
                nc.scalar.activation(e_t[:], s_t[:], AF.Exp)

                den = evpool.tile([LANES, 1], f32, tag="den")
                nc.vector.tensor_reduce(out=den[:], in_=e_t[:], axis=AX.X, op=ALU.add)
                rec = evpool.tile([LANES, 1], f32, tag="rec")
                nc.vector.tensor_scalar(
                    out=den[:], in0=den[:], scalar1=EPS, scalar2=None, op0=ALU.add)
                nc.vector.reciprocal(rec[:], den[:])
                nc.vector.tensor_scalar(
                    out=attn_sb[:, cb:cb + Ct], in0=e_t[:], scalar1=rec[:],
                    scalar2=None, op0=ALU.mult)

                # out = sum_j e_j * v_j  via diag(e) matmuls, then normalize
                psO = psacc.tile([D, D], f32, tag="pso")
                for j in range(Ct):
                    dg = evpool.tile([D, D], f16, tag="diag")
                    nc.vector.tensor_scalar(
                        out=dg[:], in0=ident_sb[:], scalar1=e_t[:, j:j + 1],
                        scalar2=None, op0=ALU.mult)
                    nc.tensor.matmul(
                        psO[:], lhsT=dg[:], rhs=vst[:, j * LANES:(j + 1) * LANES],
                        start=(j == 0), stop=(j == Ct - 1))
                out_t = evpool.tile([D, D], f32, tag="outt")
                nc.vector.tensor_scalar(
                    out=out_t[:], in0=psO[:], scalar1=rec[:], scalar2=None,
                    op0=ALU.mult)
                nc.sync.dma_start(
                    out=outp[t * LANES:(t + 1) * LANES, :], in_=out_t[:])

                cb += Ct

            nc.sync.dma_start(out=attn_o[:], in_=attn_sb[:])

    nc.compile()
    return nc


# ---------------------------------------------------------------------------
# entry point
# ---------------------------------------------------------------------------

def kernel(x, edge_index, edge_attr, Wq, Wk, Wv, bv, W1, b1, W2, b2):
    x = np.asarray(x, dtype=np.float32)
    edge_index = np.asarray(edge_index)
    ea = np.asarray(edge_attr, dtype=np.float32)
    Wq = np.asarray(Wq, np.float32); Wk = np.asarray(Wk, np.float32)
    Wv = np.asarray(Wv, np.float32); bv = np.asarray(bv, np.float32)
    W1 = np.asarray(W1, np.float32); b1 = np.asarray(b1, np.float32)
    W2 = np.asarray(W2, np.float32); b2 = np.asarray(b2, np.float32)

    tgt = edge_index[1].astype(np.int64)
    C, chunk_base, per_core = _schedule(tgt)
    wts = _prep_weights(Wq, Wk, Wv, bv, W1, b1, W2, b2)

    cores = []
    for c in range(NCORES):
        ids, tl, degp, order = per_core[c]
        cores.append(_layout_core(ids, tl, degp, order, C, chunk_base, ea, x, c))

    nc = build_graph(C, wts["P"])

    in_maps = []
    for c in range(NCORES):
        lay = cores[c]
        m = lay["mask"] + np.float32(wts["b2"])
        # padding slots keep NEGINF (+b2 is absorbed: NEGINF + b2 ~ NEGINF)
        m[lay["mask"] < -1e29] = NEGINF
        in_maps.append({
            "ea_T": lay["ea_T"],
            "mask": m.astype(np.float32),
            "x_T": lay["x_T"],
            "WkT_aug": wts["WkT_aug"],
            "WvT_aug": wts["WvT_aug"],
            "W1bT_s": wts["W1bT_s"],
            "Wq_l": wts["Wq_l"],
            "W1aT_s": wts["W1aT_s"],
            "b1_row": wts["b1_row"],
            "ident": np.eye(D, dtype=np.float16),
        })

    from concourse.bass_utils import run_bass_kernel_spmd
    res = run_bass_kernel_spmd(nc, in_maps, core_ids=list(range(NCORES)))
    kernel._last_result = res
    kernel._last_nc = nc
    kernel._last_in_maps = in_maps

    out = np.zeros((N_NODES, D), dtype=np.float32)
    attn = np.zeros(N_EDGES, dtype=np.float32)
    for c in range(NCORES):
        lay = cores[c]
        r = res.results[c]
        dev_out = r["outp"]          # [NPAD, 128] in sorted-node order
        dev_attn = r["attn_o"]       # [128, TC]
        order = lay["order"]
        sel = order < NPC
        out[c * NPC + order[sel]] = dev_out[sel]
        attn[lay["ids"]] = dev_attn[lay["lane"], lay["chunk"]]

    return out, attn


if __name__ == "__main__":
    pass
